# revision 5
# baseline (speedup 1.0000x reference)
"""GatingAttention (AlphaFold-style) Trainium2 kernel.

B=256 batches sharded across 8 NeuronCores (32/core, processed as 16
pairs).  All activations are kept feature-major on-chip so every matmul
contracts over the partition dim; fp32r matmuls run at full PE rate.

Layout notes (per core, per pair p of batches (b0, b1)):
  qdT/mdT   [2, 128, 512]   f-major activations, free dim = [b | s]
  qT2/kT2/gateT [hc-tile 2][128, 512]  head-feature-major projections
  logitsT   [sk-chunk 128, 256] per (h, c, b)  -- K=32 row-packed MMs
  expT      bf16, multiplied by resident exp(nonbatched_bias)
  waT       [hc 128, 256] per (head-group, b) -- M=32 col-packed MMs,
            denominator matmul uses exp(bias)-column lhsT (softmax scale
            invariance moves the batched bias into V and the denom)
  outT      [o-tile 2][128, 512] -> host undoes the transpose
"""
import numpy as np
import ml_dtypes
from contextlib import ExitStack

import concourse.bass as bass
import concourse.tile as tile
from concourse import bacc, mybir
from concourse.bass_utils import run_bass_kernel_spmd

dt = mybir.dt

N_CORES = 8
B, S, A, M, H, OUT = 256, 256, 256, 256, 8, 256
KD = VD = 32
BC = B // N_CORES          # 32 batches per core
NPAIR = BC // 2            # 16 pairs

_CACHE = {}


def build_nc(npair=NPAIR, num_devices=N_CORES):
    f32, f32r, bf16 = dt.float32, dt.float32r, dt.bfloat16
    nc = bacc.Bacc("TRN2", target_bir_lowering=False, debug=False,
                   num_devices=num_devices)

    def inp(name, shape, d):
        return nc.dram_tensor(name, shape, d, kind="ExternalInput").ap()

    qdT = inp("qdT", [npair, 2, 128, 512], f32r)
    mdT = inp("mdT", [npair, 2, 128, 512], f32r)
    wq = inp("wq", [2, 128, 256], f32r)
    wk = inp("wk", [2, 128, 256], f32r)
    wg = inp("wg", [2, 128, 256], f32r)
    wv = inp("wv", [2, 128, 256], f32r)
    wo = inp("wo", [2, 128, 256], f32r)
    expb1 = inp("expb1", [128, npair * 4], f32)        # col = p*4 + c*2 + b
    expb32 = inp("expb32", [128, npair * 128], bf16)   # col = (p*4+c*2+b)*32 + j
    expnb = inp("expnb", [H, 2, 128, 512], bf16)
    gb = inp("gb", [2, 128, 1], f32)
    ob = inp("ob", [2, 128, 1], f32)
    outT = nc.dram_tensor("outT", [npair, 2, 128, 512], f32,
                          kind="ExternalOutput").ap()

    with tile.TileContext(nc) as tc, ExitStack() as ctx:
        const = ctx.enter_context(tc.tile_pool(name="const", bufs=1))

        def resident(ap, d, tag):
            t = const.tile(list(ap.shape), d, tag=tag)
            nc.sync.dma_start(t[:], ap)
            return t

        wq_t = [resident(wq[c], f32r, f"wq{c}") for c in range(2)]
        wk_t = [resident(wk[c], f32r, f"wk{c}") for c in range(2)]
        wg_t = [resident(wg[c], f32r, f"wg{c}") for c in range(2)]
        wv_t = [resident(wv[c], f32r, f"wv{c}") for c in range(2)]
        wo_t = [resident(wo[c], f32r, f"wo{c}") for c in range(2)]
        expb1_t = resident(expb1, f32, "expb1")
        expb32_t = resident(expb32, bf16, "expb32")
        expnb_t = [[resident(expnb[h, c], bf16, f"expnb{h}_{c}")
                    for c in range(2)] for h in range(H)]
        gb_t = [resident(gb[c], f32, f"gb{c}") for c in range(2)]
        ob_t = [resident(ob[c], f32, f"ob{c}") for c in range(2)]

        io = ctx.enter_context(tc.tile_pool(name="io", bufs=2))
        proj = ctx.enter_context(tc.tile_pool(name="proj", bufs=2))
        vpool = ctx.enter_context(tc.tile_pool(name="vpool", bufs=2))
        exgp = ctx.enter_context(tc.tile_pool(name="exgp", bufs=18))
        gwap = ctx.enter_context(tc.tile_pool(name="gwap", bufs=2))
        smallp = ctx.enter_context(tc.tile_pool(name="smallp", bufs=3))
        outp = ctx.enter_context(tc.tile_pool(name="outp", bufs=2))

        pp = ctx.enter_context(tc.tile_pool(name="pp", bufs=2, space="PSUM"))
        lgp = ctx.enter_context(tc.tile_pool(name="lgp", bufs=3, space="PSUM"))
        wap = ctx.enter_context(tc.tile_pool(name="wap", bufs=1, space="PSUM"))
        dnp = ctx.enter_context(tc.tile_pool(name="dnp", bufs=1, space="PSUM"))
        op = ctx.enter_context(tc.tile_pool(name="op", bufs=1, space="PSUM"))

        nexp = 0
        for p in range(npair):
            qd = []
            md = []
            for c in range(2):
                t = io.tile([128, 512], f32r, tag=f"qd{c}")
                nc.sync.dma_start(t[:], qdT[p, c])
                qd.append(t)
                t = io.tile([128, 512], f32r, tag=f"md{c}")
                nc.sync.dma_start(t[:], mdT[p, c])
                md.append(t)

            # ---- projections (N=512, both batches packed) ----
            qT2, kT2, gate = [], [], []
            for mt in range(2):
                ps = pp.tile([128, 512], f32, tag="projps")
                for kc in range(2):
                    nc.tensor.matmul(ps[:], wq_t[kc][:, mt * 128:(mt + 1) * 128],
                                     qd[kc][:], start=kc == 0, stop=kc == 1)
                t = proj.tile([128, 512], f32r, tag=f"qT{mt}")
                nc.vector.tensor_copy(t[:], ps[:])
                qT2.append(t)
            for mt in range(2):
                ps = pp.tile([128, 512], f32, tag="projps")
                for kc in range(2):
                    nc.tensor.matmul(ps[:], wk_t[kc][:, mt * 128:(mt + 1) * 128],
                                     md[kc][:], start=kc == 0, stop=kc == 1)
                t = proj.tile([128, 512], f32r, tag=f"kT{mt}")
                nc.vector.tensor_copy(t[:], ps[:])
                kT2.append(t)
            for mt in range(2):
                ps = pp.tile([128, 512], f32, tag="projps")
                for kc in range(2):
                    nc.tensor.matmul(ps[:], wg_t[kc][:, mt * 128:(mt + 1) * 128],
                                     qd[kc][:], start=kc == 0, stop=kc == 1)
                t = proj.tile([128, 512], f32, tag=f"gate{mt}")
                nc.scalar.activation(t[:], ps[:],
                                     mybir.ActivationFunctionType.Sigmoid,
                                     bias=gb_t[mt][:, 0:1])
                gate.append(t)

            # ---- v projection, scaled by exp(bias) ----
            # vp[b][c] : [sk 128, hc 256] bf16
            vp = [[None, None], [None, None]]
            for b in range(2):
                for c in range(2):
                    ps = pp.tile([128, 256], f32, tag="projps")
                    for kc in range(2):
                        nc.tensor.matmul(
                            ps[:],
                            md[kc][:, b * 256 + c * 128: b * 256 + (c + 1) * 128],
                            wv_t[kc][:], start=kc == 0, stop=kc == 1)
                    t = vpool.tile([128, 256], bf16, tag=f"vp{b}{c}")
                    col = p * 4 + c * 2 + b
                    nc.vector.tensor_scalar_mul(t[:], ps[:],
                                                expb1_t[:, col:col + 1])
                    vp[b][c] = t

            # ---- logits (K=32 row-packed) + exp + nb multiply ----
            exg = [[None, None] for _ in range(H)]
            for c in range(2):
                for h in range(H):
                    ht, hr = h // 4, h % 4
                    lg = lgp.tile([128, 512], f32, tag="lg")
                    for b in range(2):
                        nc.tensor.matmul(
                            lg[:, b * 256:(b + 1) * 256],
                            kT2[ht][hr * 32:(hr + 1) * 32,
                                    b * 256 + c * 128: b * 256 + (c + 1) * 128],
                            qT2[ht][hr * 32:(hr + 1) * 32, b * 256:(b + 1) * 256],
                            start=True, stop=True, tile_position=(hr * 32, 0))
                    e = exgp.tile([128, 512], bf16, tag="exg")
                    nc.scalar.activation(e[:], lg[:],
                                         mybir.ActivationFunctionType.Exp)
                    eng = nc.vector if nexp % 8 < 5 else nc.gpsimd
                    eng.tensor_mul(e[:], e[:], expnb_t[h][c][:])
                    nexp += 1
                    exg[h][c] = e

            # ---- weighted avg (M=32 col-packed) + denom + gating ----
            gwaT = []
            for hg in range(2):
                gw_tile = gwap.tile([128, 512], f32r, tag=f"gwa{hg}")
                gwaT.append(gw_tile)
            for hg in range(2):
                for b in range(2):
                    wps = wap.tile([128, 256], f32, tag="wps")
                    dps = dnp.tile([128, 256], f32, tag="dps")
                    for h4 in range(4):
                        h = hg * 4 + h4
                        for c in range(2):
                            nc.tensor.matmul(
                                wps[h4 * 32:(h4 + 1) * 32, :],
                                vp[b][c][:, h * 32:(h + 1) * 32],
                                exg[h][c][:, b * 256:(b + 1) * 256],
                                start=c == 0, stop=c == 1,
                                tile_position=(0, h4 * 32))
                    for h4 in range(4):
                        h = hg * 4 + h4
                        for c in range(2):
                            col = (p * 4 + c * 2 + b) * 32
                            nc.tensor.matmul(
                                dps[h4 * 32:(h4 + 1) * 32, :],
                                expb32_t[:, col:col + 32],
                                exg[h][c][:, b * 256:(b + 1) * 256],
                                start=c == 0, stop=c == 1,
                                tile_position=(0, h4 * 32))
                    rec = smallp.tile([128, 256], f32, tag="rec")
                    nc.vector.reciprocal(rec[:], dps[:])
                    gr = smallp.tile([128, 256], f32, tag="gr")
                    nc.gpsimd.tensor_mul(gr[:], gate[hg][:, b * 256:(b + 1) * 256],
                                         rec[:])
                    nc.vector.tensor_mul(gwaT[hg][:, b * 256:(b + 1) * 256],
                                         wps[:], gr[:])

            # ---- output projection (outT layout) + bias ----
            for mt in range(2):
                ps = op.tile([128, 512], f32, tag="ops")
                for kc in range(2):
                    nc.tensor.matmul(ps[:], wo_t[kc][:, mt * 128:(mt + 1) * 128],
                                     gwaT[kc][:], start=kc == 0, stop=kc == 1)
                o = outp.tile([128, 512], f32, tag=f"out{mt}")
                nc.vector.tensor_scalar_add(o[:], ps[:], ob_t[mt][:, 0:1])
                nc.sync.dma_start(outT[p, mt], o[:])

    nc.compile()
    return nc


def prep_shared(query_w, key_w, value_w, gating_w, gating_b, output_w,
                output_b, nonbatched_bias):
    f32 = np.float32
    bf16 = ml_dtypes.bfloat16
    wq = (query_w.reshape(A, H * KD) * KD ** -0.5).astype(f32).reshape(2, 128, 256)
    wk = key_w.reshape(M, H * KD).astype(f32).reshape(2, 128, 256)
    wv = value_w.reshape(M, H * VD).astype(f32).reshape(2, 128, 256)
    wg = gating_w.reshape(A, H * VD).astype(f32).reshape(2, 128, 256)
    wo = output_w.reshape(H * VD, OUT).astype(f32).reshape(2, 128, 256)
    enb = np.exp(nonbatched_bias.astype(f32)).transpose(0, 2, 1)  # [H, sk, sq]
    enb = np.ascontiguousarray(enb).reshape(H, 2, 128, 256)
    enb = np.tile(enb, (1, 1, 1, 2)).astype(bf16)                 # [H, 2, 128, 512]
    gbv = gating_b.reshape(H * VD).astype(f32).reshape(2, 128, 1)
    obv = output_b.astype(f32).reshape(2, 128, 1)
    return dict(wq=np.ascontiguousarray(wq), wk=np.ascontiguousarray(wk),
                wv=np.ascontiguousarray(wv), wg=np.ascontiguousarray(wg),
                wo=np.ascontiguousarray(wo), expnb=np.ascontiguousarray(enb),
                gb=np.ascontiguousarray(gbv), ob=np.ascontiguousarray(obv))


def prep_core(q_c, m_c, bias_c, npair=NPAIR):
    """q_c, m_c: [2*npair, S, F]; bias_c: [2*npair, S]."""
    f32 = np.float32
    bf16 = ml_dtypes.bfloat16

    def tr(x):
        x = x.transpose(0, 2, 1)                       # [nb, f, s]
        x = x.reshape(npair, 2, 2, 128, 256)           # [p, b, fc, 128, s]
        x = x.transpose(0, 2, 3, 1, 4)                 # [p, fc, 128, b, s]
        return np.ascontiguousarray(x.reshape(npair, 2, 128, 512).astype(f32))

    eb = np.exp(bias_c.astype(f32))                    # [nb, sk]
    e1 = eb.reshape(npair, 2, 2, 128).transpose(3, 0, 2, 1)  # [128, p, c, b]
    e1 = np.ascontiguousarray(e1.reshape(128, npair * 4))
    e32 = np.ascontiguousarray(np.repeat(e1, 32, axis=1)).astype(bf16)
    return dict(qdT=tr(q_c), mdT=tr(m_c), expb1=e1, expb32=e32)


def unshard_out(oT, npair=NPAIR):
    """oT: [npair, 2, 128, 512] -> [2*npair, S, OUT]."""
    y = oT.reshape(npair, 2, 128, 2, 256)              # [p, mt, op, b, s]
    y = y.transpose(0, 3, 1, 2, 4)                     # [p, b, mt, op, s]
    y = y.reshape(npair * 2, 256, 256)                 # [nb, o, s]
    return np.ascontiguousarray(y.transpose(0, 2, 1))  # [nb, s, o]


def kernel(q_data, m_data, bias, nonbatched_bias, query_w, key_w, value_w,
           gating_w, gating_b, output_w, output_b):
    if "nc" not in _CACHE:
        _CACHE["nc"] = build_nc()
    nc = _CACHE["nc"]

    shared = prep_shared(np.asarray(query_w), np.asarray(key_w),
                         np.asarray(value_w), np.asarray(gating_w),
                         np.asarray(gating_b), np.asarray(output_w),
                         np.asarray(output_b), np.asarray(nonbatched_bias))
    q_data = np.asarray(q_data)
    m_data = np.asarray(m_data)
    bias2 = np.asarray(bias).reshape(B, S)

    in_maps = []
    for c in range(N_CORES):
        sl = slice(c * BC, (c + 1) * BC)
        im = dict(shared)
        im.update(prep_core(q_data[sl], m_data[sl], bias2[sl]))
        in_maps.append(im)

    res = run_bass_kernel_spmd(nc, in_maps, list(range(N_CORES)))
    outs = [unshard_out(res.results[c]["outT"]) for c in range(N_CORES)]
    return np.concatenate(outs, axis=0).astype(np.float32)


# revision 7
# speedup vs baseline: 7034.4832x; 7034.4832x over previous
"""GatingAttention (AlphaFold-style) Trainium2 kernel.

B=256 batches sharded across 8 NeuronCores (32/core, processed as 16
pairs).  All activations are kept feature-major on-chip so every matmul
contracts over the partition dim; fp32r matmuls run at full PE rate.

Layout notes (per core, per pair p of batches (b0, b1)):
  qdT/mdT   [2, 128, 512]   f-major activations, free dim = [b | s]
  qT2/kT2/gateT [hc-tile 2][128, 512]  head-feature-major projections
  logitsT   [sk-chunk 128, 256] per (h, c, b)  -- K=32 row-packed MMs
  expT      bf16, multiplied by resident exp(nonbatched_bias)
  waT       [hc 128, 256] per (head-group, b) -- M=32 col-packed MMs,
            denominator matmul uses exp(bias)-column lhsT (softmax scale
            invariance moves the batched bias into V and the denom)
  outT      [o-tile 2][128, 512] -> host undoes the transpose
"""
import numpy as np
import ml_dtypes
from contextlib import ExitStack

import concourse.bass as bass
import concourse.tile as tile
from concourse import bacc, mybir
from concourse.bass_utils import run_bass_kernel_spmd

dt = mybir.dt

N_CORES = 8
B, S, A, M, H, OUT = 256, 256, 256, 256, 8, 256
KD = VD = 32
BC = B // N_CORES          # 32 batches per core
NPAIR = BC // 2            # 16 pairs

_CACHE = {}


def build_nc(npair=NPAIR, num_devices=N_CORES, reps=1):
    f32, f32r, bf16 = dt.float32, dt.float32r, dt.bfloat16
    nc = bacc.Bacc("TRN2", target_bir_lowering=False, debug=False,
                   num_devices=num_devices)

    def inp(name, shape, d):
        return nc.dram_tensor(name, shape, d, kind="ExternalInput").ap()

    qdT = inp("qdT", [npair, 2, 128, 512], f32r)
    mdT = inp("mdT", [npair, 2, 128, 512], f32r)
    wq = inp("wq", [2, 128, 256], f32r)
    wk = inp("wk", [2, 128, 256], f32r)
    wg = inp("wg", [2, 128, 256], f32r)
    wv = inp("wv", [2, 128, 256], f32r)
    wo = inp("wo", [2, 128, 256], f32r)
    expb1 = inp("expb1", [128, npair * 4], f32)        # col = p*4 + c*2 + b
    expb32 = inp("expb32", [128, npair * 128], bf16)   # col = (p*4+c*2+b)*32 + j
    expnb = inp("expnb", [H, 2, 128, 512], bf16)
    gb = inp("gb", [2, 128, 1], f32)
    ob = inp("ob", [2, 128, 1], f32)
    outT = nc.dram_tensor("outT", [npair, 2, 128, 512], f32,
                          kind="ExternalOutput").ap()

    with tile.TileContext(nc) as tc, ExitStack() as ctx:
        const = ctx.enter_context(tc.tile_pool(name="const", bufs=1))

        def resident(ap, d, tag):
            t = const.tile(list(ap.shape), d, tag=tag)
            nc.sync.dma_start(t[:], ap)
            return t

        wq_t = [resident(wq[c], f32r, f"wq{c}") for c in range(2)]
        wk_t = [resident(wk[c], f32r, f"wk{c}") for c in range(2)]
        wg_t = [resident(wg[c], f32r, f"wg{c}") for c in range(2)]
        wv_t = [resident(wv[c], f32r, f"wv{c}") for c in range(2)]
        wo_t = [resident(wo[c], f32r, f"wo{c}") for c in range(2)]
        expb1_t = resident(expb1, f32, "expb1")
        expb32_t = resident(expb32, bf16, "expb32")
        expnb_t = [[resident(expnb[h, c], bf16, f"expnb{h}_{c}")
                    for c in range(2)] for h in range(H)]
        gb_t = [resident(gb[c], f32, f"gb{c}") for c in range(2)]
        ob_t = [resident(ob[c], f32, f"ob{c}") for c in range(2)]

        io = ctx.enter_context(tc.tile_pool(name="io", bufs=2))
        proj = ctx.enter_context(tc.tile_pool(name="proj", bufs=2))
        vpool = ctx.enter_context(tc.tile_pool(name="vpool", bufs=2))
        exgp = ctx.enter_context(tc.tile_pool(name="exgp", bufs=18))
        gwap = ctx.enter_context(tc.tile_pool(name="gwap", bufs=2))
        smallp = ctx.enter_context(tc.tile_pool(name="smallp", bufs=3))
        outp = ctx.enter_context(tc.tile_pool(name="outp", bufs=2))

        pp = ctx.enter_context(tc.tile_pool(name="pp", bufs=2, space="PSUM"))
        lgp = ctx.enter_context(tc.tile_pool(name="lgp", bufs=3, space="PSUM"))
        wap = ctx.enter_context(tc.tile_pool(name="wap", bufs=1, space="PSUM"))
        dnp = ctx.enter_context(tc.tile_pool(name="dnp", bufs=1, space="PSUM"))
        op = ctx.enter_context(tc.tile_pool(name="op", bufs=1, space="PSUM"))

        rep_ctx = tc.For_i(0, reps, 1) if reps > 1 else None
        if rep_ctx is not None:
            ctx.enter_context(rep_ctx)
        nexp = 0
        for p in range(npair):
            qd = []
            md = []
            for c in range(2):
                t = io.tile([128, 512], f32r, tag=f"qd{c}")
                nc.sync.dma_start(t[:], qdT[p, c])
                qd.append(t)
                t = io.tile([128, 512], f32r, tag=f"md{c}")
                nc.sync.dma_start(t[:], mdT[p, c])
                md.append(t)

            # ---- projections (N=512, both batches packed) ----
            qT2, kT2, gate = [], [], []
            for mt in range(2):
                ps = pp.tile([128, 512], f32, tag="projps")
                for kc in range(2):
                    nc.tensor.matmul(ps[:], wq_t[kc][:, mt * 128:(mt + 1) * 128],
                                     qd[kc][:], start=kc == 0, stop=kc == 1)
                t = proj.tile([128, 512], f32r, tag=f"qT{mt}")
                nc.vector.tensor_copy(t[:], ps[:])
                qT2.append(t)
            for mt in range(2):
                ps = pp.tile([128, 512], f32, tag="projps")
                for kc in range(2):
                    nc.tensor.matmul(ps[:], wk_t[kc][:, mt * 128:(mt + 1) * 128],
                                     md[kc][:], start=kc == 0, stop=kc == 1)
                t = proj.tile([128, 512], f32r, tag=f"kT{mt}")
                nc.vector.tensor_copy(t[:], ps[:])
                kT2.append(t)
            for mt in range(2):
                ps = pp.tile([128, 512], f32, tag="projps")
                for kc in range(2):
                    nc.tensor.matmul(ps[:], wg_t[kc][:, mt * 128:(mt + 1) * 128],
                                     qd[kc][:], start=kc == 0, stop=kc == 1)
                t = proj.tile([128, 512], f32, tag=f"gate{mt}")
                nc.scalar.activation(t[:], ps[:],
                                     mybir.ActivationFunctionType.Sigmoid,
                                     bias=gb_t[mt][:, 0:1])
                gate.append(t)

            # ---- v projection, scaled by exp(bias) ----
            # vp[b][c] : [sk 128, hc 256] bf16
            vp = [[None, None], [None, None]]
            for b in range(2):
                for c in range(2):
                    ps = pp.tile([128, 256], f32, tag="projps")
                    for kc in range(2):
                        nc.tensor.matmul(
                            ps[:],
                            md[kc][:, b * 256 + c * 128: b * 256 + (c + 1) * 128],
                            wv_t[kc][:], start=kc == 0, stop=kc == 1)
                    t = vpool.tile([128, 256], bf16, tag=f"vp{b}{c}")
                    col = p * 4 + c * 2 + b
                    nc.vector.tensor_scalar_mul(t[:], ps[:],
                                                expb1_t[:, col:col + 1])
                    vp[b][c] = t

            # ---- logits (K=32 row-packed) + exp + nb multiply ----
            exg = [[None, None] for _ in range(H)]
            for c in range(2):
                for h in range(H):
                    ht, hr = h // 4, h % 4
                    lg = lgp.tile([128, 512], f32, tag="lg")
                    for b in range(2):
                        nc.tensor.matmul(
                            lg[:, b * 256:(b + 1) * 256],
                            kT2[ht][hr * 32:(hr + 1) * 32,
                                    b * 256 + c * 128: b * 256 + (c + 1) * 128],
                            qT2[ht][hr * 32:(hr + 1) * 32, b * 256:(b + 1) * 256],
                            start=True, stop=True, tile_position=(hr * 32, 0))
                    e = exgp.tile([128, 512], bf16, tag="exg")
                    nc.scalar.activation(e[:], lg[:],
                                         mybir.ActivationFunctionType.Exp)
                    eng = nc.vector if nexp % 8 < 5 else nc.gpsimd
                    eng.tensor_mul(e[:], e[:], expnb_t[h][c][:])
                    nexp += 1
                    exg[h][c] = e

            # ---- weighted avg (M=32 col-packed) + denom + gating ----
            gwaT = []
            for hg in range(2):
                gw_tile = gwap.tile([128, 512], f32r, tag=f"gwa{hg}")
                gwaT.append(gw_tile)
            for hg in range(2):
                for b in range(2):
                    wps = wap.tile([128, 256], f32, tag="wps")
                    dps = dnp.tile([128, 256], f32, tag="dps")
                    for h4 in range(4):
                        h = hg * 4 + h4
                        for c in range(2):
                            nc.tensor.matmul(
                                wps[h4 * 32:(h4 + 1) * 32, :],
                                vp[b][c][:, h * 32:(h + 1) * 32],
                                exg[h][c][:, b * 256:(b + 1) * 256],
                                start=c == 0, stop=c == 1,
                                tile_position=(0, h4 * 32))
                    for h4 in range(4):
                        h = hg * 4 + h4
                        for c in range(2):
                            col = (p * 4 + c * 2 + b) * 32
                            nc.tensor.matmul(
                                dps[h4 * 32:(h4 + 1) * 32, :],
                                expb32_t[:, col:col + 32],
                                exg[h][c][:, b * 256:(b + 1) * 256],
                                start=c == 0, stop=c == 1,
                                tile_position=(0, h4 * 32))
                    rec = smallp.tile([128, 256], f32, tag="rec")
                    nc.vector.reciprocal(rec[:], dps[:])
                    gr = smallp.tile([128, 256], f32, tag="gr")
                    nc.gpsimd.tensor_mul(gr[:], gate[hg][:, b * 256:(b + 1) * 256],
                                         rec[:])
                    nc.vector.tensor_mul(gwaT[hg][:, b * 256:(b + 1) * 256],
                                         wps[:], gr[:])

            # ---- output projection (outT layout) + bias ----
            for mt in range(2):
                ps = op.tile([128, 512], f32, tag="ops")
                for kc in range(2):
                    nc.tensor.matmul(ps[:], wo_t[kc][:, mt * 128:(mt + 1) * 128],
                                     gwaT[kc][:], start=kc == 0, stop=kc == 1)
                o = outp.tile([128, 512], f32, tag=f"out{mt}")
                nc.vector.tensor_scalar_add(o[:], ps[:], ob_t[mt][:, 0:1])
                nc.sync.dma_start(outT[p, mt], o[:])

    nc.compile()
    return nc


def prep_shared(query_w, key_w, value_w, gating_w, gating_b, output_w,
                output_b, nonbatched_bias):
    f32 = np.float32
    bf16 = ml_dtypes.bfloat16
    wq = (query_w.reshape(A, H * KD) * KD ** -0.5).astype(f32).reshape(2, 128, 256)
    wk = key_w.reshape(M, H * KD).astype(f32).reshape(2, 128, 256)
    wv = value_w.reshape(M, H * VD).astype(f32).reshape(2, 128, 256)
    wg = gating_w.reshape(A, H * VD).astype(f32).reshape(2, 128, 256)
    wo = output_w.reshape(H * VD, OUT).astype(f32).reshape(2, 128, 256)
    enb = np.exp(nonbatched_bias.astype(f32)).transpose(0, 2, 1)  # [H, sk, sq]
    enb = np.ascontiguousarray(enb).reshape(H, 2, 128, 256)
    enb = np.tile(enb, (1, 1, 1, 2)).astype(bf16)                 # [H, 2, 128, 512]
    gbv = gating_b.reshape(H * VD).astype(f32).reshape(2, 128, 1)
    obv = output_b.astype(f32).reshape(2, 128, 1)
    return dict(wq=np.ascontiguousarray(wq), wk=np.ascontiguousarray(wk),
                wv=np.ascontiguousarray(wv), wg=np.ascontiguousarray(wg),
                wo=np.ascontiguousarray(wo), expnb=np.ascontiguousarray(enb),
                gb=np.ascontiguousarray(gbv), ob=np.ascontiguousarray(obv))


def prep_core(q_c, m_c, bias_c, npair=NPAIR):
    """q_c, m_c: [2*npair, S, F]; bias_c: [2*npair, S]."""
    f32 = np.float32
    bf16 = ml_dtypes.bfloat16

    def tr(x):
        x = x.transpose(0, 2, 1)                       # [nb, f, s]
        x = x.reshape(npair, 2, 2, 128, 256)           # [p, b, fc, 128, s]
        x = x.transpose(0, 2, 3, 1, 4)                 # [p, fc, 128, b, s]
        return np.ascontiguousarray(x.reshape(npair, 2, 128, 512).astype(f32))

    eb = np.exp(bias_c.astype(f32))                    # [nb, sk]
    e1 = eb.reshape(npair, 2, 2, 128).transpose(3, 0, 2, 1)  # [128, p, c, b]
    e1 = np.ascontiguousarray(e1.reshape(128, npair * 4))
    e32 = np.ascontiguousarray(np.repeat(e1, 32, axis=1)).astype(bf16)
    return dict(qdT=tr(q_c), mdT=tr(m_c), expb1=e1, expb32=e32)


def unshard_out(oT, npair=NPAIR):
    """oT: [npair, 2, 128, 512] -> [2*npair, S, OUT]."""
    y = oT.reshape(npair, 2, 128, 2, 256)              # [p, mt, op, b, s]
    y = y.transpose(0, 3, 1, 2, 4)                     # [p, b, mt, op, s]
    y = y.reshape(npair * 2, 256, 256)                 # [nb, o, s]
    return np.ascontiguousarray(y.transpose(0, 2, 1))  # [nb, s, o]


def kernel(q_data, m_data, bias, nonbatched_bias, query_w, key_w, value_w,
           gating_w, gating_b, output_w, output_b):
    if "nc" not in _CACHE:
        _CACHE["nc"] = build_nc()
    nc = _CACHE["nc"]

    shared = prep_shared(np.asarray(query_w), np.asarray(key_w),
                         np.asarray(value_w), np.asarray(gating_w),
                         np.asarray(gating_b), np.asarray(output_w),
                         np.asarray(output_b), np.asarray(nonbatched_bias))
    q_data = np.asarray(q_data)
    m_data = np.asarray(m_data)
    bias2 = np.asarray(bias).reshape(B, S)

    in_maps = []
    for c in range(N_CORES):
        sl = slice(c * BC, (c + 1) * BC)
        im = dict(shared)
        im.update(prep_core(q_data[sl], m_data[sl], bias2[sl]))
        in_maps.append(im)

    res = run_bass_kernel_spmd(nc, in_maps, list(range(N_CORES)))
    outs = [unshard_out(res.results[c]["outT"]) for c in range(N_CORES)]
    return np.concatenate(outs, axis=0).astype(np.float32)


# revision 9
# speedup vs baseline: 7974.2258x; 1.1336x over previous
"""GatingAttention (AlphaFold-style) Trainium2 kernel.

B=256 batches sharded across 8 NeuronCores (32/core, processed as 16
pairs).  All activations are kept feature-major on-chip so every matmul
contracts over the partition dim; fp32r matmuls run at full PE rate.

Layout notes (per core, per pair p of batches (b0, b1)):
  qdT/mdT   [2, 128, 512]   f-major activations, free dim = [b | s]
  qT2/kT2/gateT [hc-tile 2][128, 512]  head-feature-major projections
  logitsT   [sk-chunk 128, 256] per (h, c, b)  -- K=32 row-packed MMs
  expT      bf16, multiplied by resident exp(nonbatched_bias)
  waT       [hc 128, 256] per (head-group, b) -- M=32 col-packed MMs,
            denominator matmul uses exp(bias)-column lhsT (softmax scale
            invariance moves the batched bias into V and the denom)
  outT      [o-tile 2][128, 512] -> host undoes the transpose
"""
import numpy as np
import ml_dtypes
from contextlib import ExitStack

import concourse.bass as bass
import concourse.tile as tile
from concourse import bacc, mybir
from concourse.bass_utils import run_bass_kernel_spmd

dt = mybir.dt

N_CORES = 8
B, S, A, M, H, OUT = 256, 256, 256, 256, 8, 256
KD = VD = 32
BC = B // N_CORES          # 32 batches per core
NPAIR = BC // 2            # 16 pairs

_CACHE = {}


def build_nc(npair=NPAIR, num_devices=N_CORES, reps=1):
    f32, f32r, bf16 = dt.float32, dt.float32r, dt.bfloat16
    nc = bacc.Bacc("TRN2", target_bir_lowering=False, debug=False,
                   num_devices=num_devices)

    def inp(name, shape, d):
        return nc.dram_tensor(name, shape, d, kind="ExternalInput").ap()

    qdT = inp("qdT", [npair, 2, 128, 512], f32r)
    mdT = inp("mdT", [npair, 2, 128, 512], f32r)
    wq = inp("wq", [2, 128, 256], f32r)
    wk = inp("wk", [2, 128, 256], f32r)
    wg = inp("wg", [2, 128, 256], f32r)
    wv = inp("wv", [2, 128, 256], f32r)
    wo = inp("wo", [2, 128, 256], f32r)
    expb1 = inp("expb1", [128, npair * 4], f32)        # col = p*4 + c*2 + b
    expb32 = inp("expb32", [128, npair * 128], bf16)   # col = (p*4+c*2+b)*32 + j
    expnb = inp("expnb", [H, 2, 128, 512], bf16)
    gb = inp("gb", [2, 128, 1], f32)
    ob = inp("ob", [2, 128, 1], f32)
    outT = nc.dram_tensor("outT", [npair, 2, 128, 512], f32,
                          kind="ExternalOutput").ap()

    with tile.TileContext(nc) as tc, ExitStack() as ctx:
        const = ctx.enter_context(tc.tile_pool(name="const", bufs=1))

        def resident(ap, d, tag):
            t = const.tile(list(ap.shape), d, tag=tag)
            nc.sync.dma_start(t[:], ap)
            return t

        wq_t = [resident(wq[c], f32r, f"wq{c}") for c in range(2)]
        wk_t = [resident(wk[c], f32r, f"wk{c}") for c in range(2)]
        wg_t = [resident(wg[c], f32r, f"wg{c}") for c in range(2)]
        wv_t = [resident(wv[c], f32r, f"wv{c}") for c in range(2)]
        wo_t = [resident(wo[c], f32r, f"wo{c}") for c in range(2)]
        expb1_t = resident(expb1, f32, "expb1")
        expb32_t = resident(expb32, bf16, "expb32")
        expnb_t = [[resident(expnb[h, c], bf16, f"expnb{h}_{c}")
                    for c in range(2)] for h in range(H)]
        gb_t = [resident(gb[c], f32, f"gb{c}") for c in range(2)]
        ob_t = [resident(ob[c], f32, f"ob{c}") for c in range(2)]

        io = ctx.enter_context(tc.tile_pool(name="io", bufs=3))
        proj = ctx.enter_context(tc.tile_pool(name="proj", bufs=2))
        vpool = ctx.enter_context(tc.tile_pool(name="vpool", bufs=3))
        exgp = ctx.enter_context(tc.tile_pool(name="exgp", bufs=18))
        gwap = ctx.enter_context(tc.tile_pool(name="gwap", bufs=2))
        smallp = ctx.enter_context(tc.tile_pool(name="smallp", bufs=4))
        outp = ctx.enter_context(tc.tile_pool(name="outp", bufs=2))

        pp = ctx.enter_context(tc.tile_pool(name="pp", bufs=2, space="PSUM"))
        lgp = ctx.enter_context(tc.tile_pool(name="lgp", bufs=4, space="PSUM"))
        wap = ctx.enter_context(tc.tile_pool(name="wap", bufs=1, space="PSUM"))
        dnp = ctx.enter_context(tc.tile_pool(name="dnp", bufs=1, space="PSUM"))
        op = pp  # out-proj shares the projection psum pool (disjoint phases)

        rep_ctx = tc.For_i(0, reps, 1) if reps > 1 else None
        if rep_ctx is not None:
            ctx.enter_context(rep_ctx)
        nexp = 0
        for p in range(npair):
            qd = []
            md = []
            for c in range(2):
                t = io.tile([128, 512], f32r, tag=f"qd{c}")
                nc.sync.dma_start(t[:], qdT[p, c])
                qd.append(t)
                t = io.tile([128, 512], f32r, tag=f"md{c}")
                nc.sync.dma_start(t[:], mdT[p, c])
                md.append(t)

            # ---- projections (N=512, both batches packed) ----
            qT2, kT2, gate = [], [], []
            for mt in range(2):
                ps = pp.tile([128, 512], f32, tag="projps")
                for kc in range(2):
                    nc.tensor.matmul(ps[:], wq_t[kc][:, mt * 128:(mt + 1) * 128],
                                     qd[kc][:], start=kc == 0, stop=kc == 1)
                t = proj.tile([128, 512], f32r, tag=f"qT{mt}")
                nc.vector.tensor_copy(t[:], ps[:])
                qT2.append(t)
            for mt in range(2):
                ps = pp.tile([128, 512], f32, tag="projps")
                for kc in range(2):
                    nc.tensor.matmul(ps[:], wk_t[kc][:, mt * 128:(mt + 1) * 128],
                                     md[kc][:], start=kc == 0, stop=kc == 1)
                t = proj.tile([128, 512], f32r, tag=f"kT{mt}")
                nc.vector.tensor_copy(t[:], ps[:])
                kT2.append(t)
            for mt in range(2):
                ps = pp.tile([128, 512], f32, tag="projps")
                for kc in range(2):
                    nc.tensor.matmul(ps[:], wg_t[kc][:, mt * 128:(mt + 1) * 128],
                                     qd[kc][:], start=kc == 0, stop=kc == 1)
                t = proj.tile([128, 512], f32, tag=f"gate{mt}")
                nc.scalar.activation(t[:], ps[:],
                                     mybir.ActivationFunctionType.Sigmoid,
                                     bias=gb_t[mt][:, 0:1])
                gate.append(t)

            # ---- v projection, scaled by exp(bias) ----
            # vp[b][c] : [sk 128, hc 256] bf16
            vp = [[None, None], [None, None]]
            for b in range(2):
                for c in range(2):
                    ps = pp.tile([128, 256], f32, tag="projps")
                    for kc in range(2):
                        nc.tensor.matmul(
                            ps[:],
                            md[kc][:, b * 256 + c * 128: b * 256 + (c + 1) * 128],
                            wv_t[kc][:], start=kc == 0, stop=kc == 1)
                    t = vpool.tile([128, 256], bf16, tag=f"vp{b}{c}")
                    col = p * 4 + c * 2 + b
                    nc.vector.tensor_scalar_mul(t[:], ps[:],
                                                expb1_t[:, col:col + 1])
                    vp[b][c] = t

            # ---- logits (K=32 row-packed) + exp + nb multiply ----
            exg = [[None, None] for _ in range(H)]
            for c in range(2):
                for h in range(H):
                    ht, hr = h // 4, h % 4
                    lg = lgp.tile([128, 512], f32, tag="lg")
                    for b in range(2):
                        nc.tensor.matmul(
                            lg[:, b * 256:(b + 1) * 256],
                            kT2[ht][hr * 32:(hr + 1) * 32,
                                    b * 256 + c * 128: b * 256 + (c + 1) * 128],
                            qT2[ht][hr * 32:(hr + 1) * 32, b * 256:(b + 1) * 256],
                            start=True, stop=True, tile_position=(hr * 32, 0))
                    e = exgp.tile([128, 512], bf16, tag="exg")
                    nc.scalar.activation(e[:], lg[:],
                                         mybir.ActivationFunctionType.Exp)
                    eng = nc.vector if nexp % 8 < 5 else nc.gpsimd
                    eng.tensor_mul(e[:], e[:], expnb_t[h][c][:])
                    nexp += 1
                    exg[h][c] = e

            # ---- weighted avg (M=32 col-packed) + denom + gating ----
            gwaT = []
            for hg in range(2):
                gw_tile = gwap.tile([128, 512], f32r, tag=f"gwa{hg}")
                gwaT.append(gw_tile)
            for hg in range(2):
                for b in range(2):
                    wps = wap.tile([128, 256], f32, tag="wps")
                    dps = dnp.tile([128, 256], f32, tag="dps")
                    for h4 in range(4):
                        h = hg * 4 + h4
                        for c in range(2):
                            nc.tensor.matmul(
                                wps[h4 * 32:(h4 + 1) * 32, :],
                                vp[b][c][:, h * 32:(h + 1) * 32],
                                exg[h][c][:, b * 256:(b + 1) * 256],
                                start=c == 0, stop=c == 1,
                                tile_position=(0, h4 * 32))
                    for h4 in range(4):
                        h = hg * 4 + h4
                        for c in range(2):
                            col = (p * 4 + c * 2 + b) * 32
                            nc.tensor.matmul(
                                dps[h4 * 32:(h4 + 1) * 32, :],
                                expb32_t[:, col:col + 32],
                                exg[h][c][:, b * 256:(b + 1) * 256],
                                start=c == 0, stop=c == 1,
                                tile_position=(0, h4 * 32))
                    rec = smallp.tile([128, 256], f32, tag="rec")
                    nc.vector.reciprocal(rec[:], dps[:])
                    gr = smallp.tile([128, 256], f32, tag="gr")
                    nc.gpsimd.tensor_mul(gr[:], gate[hg][:, b * 256:(b + 1) * 256],
                                         rec[:])
                    nc.vector.tensor_mul(gwaT[hg][:, b * 256:(b + 1) * 256],
                                         wps[:], gr[:])

            # ---- output projection (outT layout) + bias ----
            for mt in range(2):
                ps = op.tile([128, 512], f32, tag="projps")
                for kc in range(2):
                    nc.tensor.matmul(ps[:], wo_t[kc][:, mt * 128:(mt + 1) * 128],
                                     gwaT[kc][:], start=kc == 0, stop=kc == 1)
                o = outp.tile([128, 512], f32, tag=f"out{mt}")
                nc.vector.tensor_scalar_add(o[:], ps[:], ob_t[mt][:, 0:1])
                nc.sync.dma_start(outT[p, mt], o[:])

    nc.compile()
    return nc


def prep_shared(query_w, key_w, value_w, gating_w, gating_b, output_w,
                output_b, nonbatched_bias):
    f32 = np.float32
    bf16 = ml_dtypes.bfloat16
    wq = (query_w.reshape(A, H * KD) * KD ** -0.5).astype(f32).reshape(2, 128, 256)
    wk = key_w.reshape(M, H * KD).astype(f32).reshape(2, 128, 256)
    wv = value_w.reshape(M, H * VD).astype(f32).reshape(2, 128, 256)
    wg = gating_w.reshape(A, H * VD).astype(f32).reshape(2, 128, 256)
    wo = output_w.reshape(H * VD, OUT).astype(f32).reshape(2, 128, 256)
    enb = np.exp(nonbatched_bias.astype(f32)).transpose(0, 2, 1)  # [H, sk, sq]
    enb = np.ascontiguousarray(enb).reshape(H, 2, 128, 256)
    enb = np.tile(enb, (1, 1, 1, 2)).astype(bf16)                 # [H, 2, 128, 512]
    gbv = gating_b.reshape(H * VD).astype(f32).reshape(2, 128, 1)
    obv = output_b.astype(f32).reshape(2, 128, 1)
    return dict(wq=np.ascontiguousarray(wq), wk=np.ascontiguousarray(wk),
                wv=np.ascontiguousarray(wv), wg=np.ascontiguousarray(wg),
                wo=np.ascontiguousarray(wo), expnb=np.ascontiguousarray(enb),
                gb=np.ascontiguousarray(gbv), ob=np.ascontiguousarray(obv))


def prep_core(q_c, m_c, bias_c, npair=NPAIR):
    """q_c, m_c: [2*npair, S, F]; bias_c: [2*npair, S]."""
    f32 = np.float32
    bf16 = ml_dtypes.bfloat16

    def tr(x):
        x = x.transpose(0, 2, 1)                       # [nb, f, s]
        x = x.reshape(npair, 2, 2, 128, 256)           # [p, b, fc, 128, s]
        x = x.transpose(0, 2, 3, 1, 4)                 # [p, fc, 128, b, s]
        return np.ascontiguousarray(x.reshape(npair, 2, 128, 512).astype(f32))

    eb = np.exp(bias_c.astype(f32))                    # [nb, sk]
    e1 = eb.reshape(npair, 2, 2, 128).transpose(3, 0, 2, 1)  # [128, p, c, b]
    e1 = np.ascontiguousarray(e1.reshape(128, npair * 4))
    e32 = np.ascontiguousarray(np.repeat(e1, 32, axis=1)).astype(bf16)
    return dict(qdT=tr(q_c), mdT=tr(m_c), expb1=e1, expb32=e32)


def unshard_out(oT, npair=NPAIR):
    """oT: [npair, 2, 128, 512] -> [2*npair, S, OUT]."""
    y = oT.reshape(npair, 2, 128, 2, 256)              # [p, mt, op, b, s]
    y = y.transpose(0, 3, 1, 2, 4)                     # [p, b, mt, op, s]
    y = y.reshape(npair * 2, 256, 256)                 # [nb, o, s]
    return np.ascontiguousarray(y.transpose(0, 2, 1))  # [nb, s, o]


def kernel(q_data, m_data, bias, nonbatched_bias, query_w, key_w, value_w,
           gating_w, gating_b, output_w, output_b):
    if "nc" not in _CACHE:
        _CACHE["nc"] = build_nc()
    nc = _CACHE["nc"]

    shared = prep_shared(np.asarray(query_w), np.asarray(key_w),
                         np.asarray(value_w), np.asarray(gating_w),
                         np.asarray(gating_b), np.asarray(output_w),
                         np.asarray(output_b), np.asarray(nonbatched_bias))
    q_data = np.asarray(q_data)
    m_data = np.asarray(m_data)
    bias2 = np.asarray(bias).reshape(B, S)

    in_maps = []
    for c in range(N_CORES):
        sl = slice(c * BC, (c + 1) * BC)
        im = dict(shared)
        im.update(prep_core(q_data[sl], m_data[sl], bias2[sl]))
        in_maps.append(im)

    res = run_bass_kernel_spmd(nc, in_maps, list(range(N_CORES)))
    outs = [unshard_out(res.results[c]["outT"]) for c in range(N_CORES)]
    return np.concatenate(outs, axis=0).astype(np.float32)
